# revision 1
# baseline (speedup 1.0000x reference)
"""Trainium2 Bass kernel for quantized-MLP-with-LoRA (nn_MixedSparseTraditionalMLP).

Strategy: data-parallel over the 8192 tokens across 8 NeuronCores (1024 tokens
per core). Each core holds the full weights, dequantizes the 4-bit codes to
fp16 on-chip (DVE cast+offset, then one broadcast-scale multiply), transposes
operands into contraction-major layout with the DMA xbar (SBUF->SBUF), and runs
both projections as fp16 matmuls with fp32 PSUM accumulation. LoRA terms and
the down-projection bias are folded into the same PSUM accumulation groups as
extra low-rank matmul steps; relu + up-bias are applied by the scalar engine on
the PSUM->SBUF copy. x2 (the hidden activation) round-trips through DRAM in
transposed layout so the down projection streams it as the stationary operand.
No collectives: the host just concatenates the 8 per-core token slices.
"""
import sys

if "/opt/trn_rl_repo" not in sys.path:
    sys.path.insert(0, "/opt/trn_rl_repo")

import numpy as np

import concourse.bass as bass
import concourse.mybir as mybir
import concourse.tile as tile
from concourse import bacc
from concourse.bass import ts, ds
from concourse.bass_utils import run_bass_kernel_spmd

F16 = mybir.dt.float16
F32 = mybir.dt.float32
I32 = mybir.dt.int32

NCORES = 8
T = 1024          # tokens per core
D = 2048
H = 8192
R = 16
P = 128
KD = D // P       # 16 k-subtiles for the up contraction
KH = H // P       # 64 k-subtiles for the down contraction
NT = T // 512     # 2 moving-operand tiles of 512 tokens
DM = 4            # down-projection d tiles of 512

TRACE = False
LAST_RESULTS = None


def _build():
    nc = bacc.Bacc("TRN2", target_bir_lowering=False, debug=False,
                   enable_asserts=False, num_devices=NCORES)

    x1c = nc.dram_tensor("x1c", [T, D], F32, kind="ExternalInput").ap()
    wupq = nc.dram_tensor("wupq", [H, D], I32, kind="ExternalInput").ap()
    sup = nc.dram_tensor("sup", [H, D // 64], F32, kind="ExternalInput").ap()
    bup = nc.dram_tensor("bup", [H], F32, kind="ExternalInput").ap()
    a1 = nc.dram_tensor("a1", [D, R], F32, kind="ExternalInput").ap()
    b1 = nc.dram_tensor("b1", [R, H], F32, kind="ExternalInput").ap()
    wdnq = nc.dram_tensor("wdnq", [D, H], I32, kind="ExternalInput").ap()
    sdn = nc.dram_tensor("sdn", [D, H // 64], F32, kind="ExternalInput").ap()
    bdn = nc.dram_tensor("bdn", [D], F32, kind="ExternalInput").ap()
    a2 = nc.dram_tensor("a2", [H, R], F32, kind="ExternalInput").ap()
    b2 = nc.dram_tensor("b2", [R, D], F32, kind="ExternalInput").ap()
    y2c = nc.dram_tensor("y2c", [T, D], F32, kind="ExternalOutput").ap()

    with tile.TileContext(nc) as tc:
        with tc.tile_pool(name="const", bufs=1) as cp, \
             tc.tile_pool(name="dram", bufs=1, space="DRAM") as dp, \
             tc.tile_pool(name="psum", bufs=4, space="PSUM") as pp, \
             tc.tile_pool(name="psum_vt", bufs=1, space="PSUM") as pvt:

            x2td = dp.tile([KH, P, T], F16)     # transposed hidden activation

            # constants that live through both phases
            sdnf = cp.tile([P, D // P, H // 64], F16, tag="sdnf")
            a2f = cp.tile([P, KH, R], F16, tag="a2f")
            b2p = cp.tile([R + 1, D], F16, tag="b2p")
            v1t = cp.tile([R + 1, T], F16, tag="v1t")
            # row R must read 1.0 (folds b_down into the lora matmul); rows
            # 0..R-1 are overwritten with vT after the up phase
            nc.any.memset(v1t[:], 1.0)

            vt_ps = [pvt.tile([R, 512], F32, tag=f"vt{i}", name=f"vt{i}")
                     for i in range(NT)]

            with tc.tile_pool(name="upc", bufs=1) as up, \
                 tc.tile_pool(name="stage", bufs=3) as sp, \
                 tc.tile_pool(name="wup", bufs=3) as wp, \
                 tc.tile_pool(name="x2s", bufs=3) as xp:

                x1t = up.tile([P, KD, T], F16, tag="x1t")
                supf = up.tile([P, H // P, D // 64], F16, tag="supf")
                a1f = up.tile([P, KD, R], F16, tag="a1f")
                b1f = up.tile([R, H], F16, tag="b1f")
                bupsb = up.tile([P, KH], F32, tag="bupsb")
                utf = up.tile([R, T], F16, tag="utf")

                # ---- prep: scales, lora mats, biases (via f32 staging tiles) ----
                stx = sp.tile([P, D], F32, tag="st32")
                v = stx[:].rearrange("p (o b) -> p o b", b=H // 64)  # [128,16,128]
                nc.sync.dma_start(v, sdn.rearrange("(o p) b -> p o b", p=P))
                nc.vector.tensor_copy(sdnf[:], v)

                stx = sp.tile([P, D], F32, tag="st32")
                v = stx[:, : KH * R].rearrange("p (o r) -> p o r", r=R)
                nc.sync.dma_start(v, a2.rearrange("(o p) r -> p o r", p=P))
                nc.vector.tensor_copy(a2f[:], v)

                stx = sp.tile([P, D], F32, tag="st32")
                nc.sync.dma_start(stx[:R, :], b2)
                nc.sync.dma_start(stx[R:R + 1, :], bdn[None, :])
                nc.vector.tensor_copy(b2p[:], stx[:R + 1, :])

                stx = sp.tile([P, D], F32, tag="st32")
                v = stx[:].rearrange("p (o b) -> p o b", b=D // 64)  # [128,64,32]
                nc.sync.dma_start(v, sup.rearrange("(o p) b -> p o b", p=P))
                nc.vector.tensor_copy(supf[:], v)

                stx = sp.tile([P, D], F32, tag="st32")
                v = stx[:, : KD * R].rearrange("p (o r) -> p o r", r=R)
                nc.sync.dma_start(v, a1.rearrange("(o p) r -> p o r", p=P))
                nc.vector.tensor_copy(a1f[:], v)

                for c in range(4):
                    stx = sp.tile([P, D], F32, tag="st32")
                    nc.sync.dma_start(stx[:R, :], b1[:, ts(c, D)])
                    nc.vector.tensor_copy(b1f[:, ts(c, D)], stx[:R, :])
                nc.sync.dma_start(bupsb[:], bup.rearrange("(o p) -> p o", p=P))

                # ---- x1 -> fp16, transposed to [d_partition, d_subtile, token] ----
                for s in range(T // P):
                    stx = sp.tile([P, D], F32, tag="st32")
                    nc.sync.dma_start(stx[:], x1c[ts(s, P), :])
                    xf = sp.tile([P, D], F16, tag="xf")
                    nc.vector.tensor_copy(xf[:], stx[:])
                    nc.sync.dma_start_transpose(x1t[:, :, ts(s, P)], xf[:])

                # ---- uT = (x1 @ A1)^T : [R, T] ----
                for tt in range(NT):
                    ups = pp.tile([R, 512], F32, tag="mm")
                    for j in range(KD):
                        nc.tensor.matmul(ups[:], a1f[:, j, :], x1t[:, j, ts(tt, 512)],
                                         start=(j == 0), stop=(j == KD - 1))
                    nc.scalar.copy(utf[:, ts(tt, 512)], ups[:])

                # ---- UP: one 128-row slab of H per step ----
                for k in range(KH):
                    qst = sp.tile([P, D], I32, tag="qst")
                    nc.sync.dma_start(qst[:], wupq[ts(k, P), :])
                    qf = sp.tile([P, D], F16, tag="qf")
                    nc.vector.tensor_scalar_add(qf[:], qst[:], -7.5)
                    nc.vector.tensor_tensor(
                        qf[:].rearrange("p (b i) -> p b i", i=64),
                        qf[:].rearrange("p (b i) -> p b i", i=64),
                        supf[:, k, :, None].to_broadcast((P, D // 64, 64)),
                        mybir.AluOpType.mult)
                    wt = wp.tile([P, KD, P], F16, tag="wupt")
                    nc.sync.dma_start_transpose(wt[:], qf[:])

                    x2sl = xp.tile([P, T], F16, tag="x2sl")
                    for tt in range(NT):
                        ps = pp.tile([P, 512], F32, tag="mm")
                        for j in range(KD):
                            nc.tensor.matmul(ps[:], wt[:, j, :], x1t[:, j, ts(tt, 512)],
                                             start=(j == 0), stop=False)
                        nc.tensor.matmul(ps[:], b1f[:, ts(k, P)], utf[:, ts(tt, 512)],
                                         start=False, stop=True)
                        nc.scalar.activation(x2sl[:, ts(tt, 512)], ps[:],
                                             mybir.ActivationFunctionType.Relu,
                                             bias=bupsb[:, k:k + 1], scale=1.0)
                        nc.tensor.matmul(vt_ps[tt][:], a2f[:, k, :], x2sl[:, ts(tt, 512)],
                                         start=(k == 0), stop=(k == KH - 1),
                                         skip_group_check=True)
                    nc.sync.dma_start(x2td[k], x2sl[:])

                for tt in range(NT):
                    nc.scalar.copy(v1t[:R, ts(tt, 512)], vt_ps[tt][:])

            # ---- DOWN: stream x2^T tiles and dequantized w_down tiles ----
            with tc.tile_pool(name="wdn", bufs=2) as wd, \
                 tc.tile_pool(name="x2r", bufs=2) as xr, \
                 tc.tile_pool(name="dstage", bufs=2) as dsp, \
                 tc.tile_pool(name="yout", bufs=2) as yp:
                for m in range(DM):
                    wdt = wd.tile([P, KH, 512], F16, tag="wdt")
                    for s in range(4):           # 128-row d slabs within the 512 tile
                        d0 = 512 * m + 128 * s
                        for c in range(8):       # 1024-wide h chunks
                            qst = dsp.tile([P, 1024], I32, tag="qst")
                            nc.sync.dma_start(qst[:], wdnq[ds(d0, P), ts(c, 1024)])
                            qf = dsp.tile([P, 1024], F16, tag="qf")
                            nc.vector.tensor_scalar_add(qf[:], qst[:], -7.5)
                            nc.vector.tensor_tensor(
                                qf[:].rearrange("p (b i) -> p b i", i=64),
                                qf[:].rearrange("p (b i) -> p b i", i=64),
                                sdnf[:, 4 * m + s, ds(16 * c, 16), None].to_broadcast(
                                    (P, 16, 64)),
                                mybir.AluOpType.mult)
                            nc.sync.dma_start_transpose(
                                wdt[:, ds(8 * c, 8), ts(s, P)], qf[:])
                    for t8 in range(T // P):
                        x2r_t = xr.tile([P, KH, P], F16, tag="x2r")
                        nc.sync.dma_start(
                            x2r_t[:], x2td[:, :, ts(t8, P)].rearrange("k p t -> p k t"))
                        ps = pp.tile([P, 512], F32, tag="mm")
                        for k in range(KH):
                            nc.tensor.matmul(ps[:], x2r_t[:, k, :], wdt[:, k, :],
                                             start=(k == 0), stop=False)
                        nc.tensor.matmul(ps[:], v1t[:, ts(t8, P)], b2p[:, ts(m, 512)],
                                         start=False, stop=True)
                        yo = yp.tile([P, 512], F32, tag="yo")
                        nc.scalar.copy(yo[:], ps[:])
                        nc.sync.dma_start(y2c[ts(t8, P), ts(m, 512)], yo[:])

    nc.compile()
    return nc


_NC = None


def kernel(x1, w_up_q, w_up_scale, b_up, w_up_lora_a, w_up_lora_b,
           w_down_q, w_down_scale, b_down, w_down_lora_a, w_down_lora_b):
    global _NC, LAST_RESULTS
    if _NC is None:
        _NC = _build()

    x1 = np.ascontiguousarray(np.asarray(x1, dtype=np.float32))
    B, S, _ = x1.shape
    xf = x1.reshape(B * S, D)
    shared = {
        "wupq": np.ascontiguousarray(np.asarray(w_up_q, dtype=np.int32)),
        "sup": np.ascontiguousarray(np.asarray(w_up_scale, dtype=np.float32)),
        "bup": np.ascontiguousarray(np.asarray(b_up, dtype=np.float32)),
        "a1": np.ascontiguousarray(np.asarray(w_up_lora_a, dtype=np.float32)),
        "b1": np.ascontiguousarray(np.asarray(w_up_lora_b, dtype=np.float32)),
        "wdnq": np.ascontiguousarray(np.asarray(w_down_q, dtype=np.int32)),
        "sdn": np.ascontiguousarray(np.asarray(w_down_scale, dtype=np.float32)),
        "bdn": np.ascontiguousarray(np.asarray(b_down, dtype=np.float32)),
        "a2": np.ascontiguousarray(np.asarray(w_down_lora_a, dtype=np.float32)),
        "b2": np.ascontiguousarray(np.asarray(w_down_lora_b, dtype=np.float32)),
    }
    in_maps = [{"x1c": np.ascontiguousarray(xf[c * T:(c + 1) * T]), **shared}
               for c in range(NCORES)]

    res = run_bass_kernel_spmd(_NC, in_maps, core_ids=list(range(NCORES)),
                               trace=TRACE)
    LAST_RESULTS = res
    out = np.concatenate([res.results[c]["y2c"] for c in range(NCORES)], axis=0)
    return out.reshape(B, S, D)

